# revision 73
# baseline (speedup 1.0000x reference)
"""Multi-head self-attention on 8 Trainium2 NeuronCores.

Problem: x[4, 2048, 1024], 16 heads x 64 dims, fused qkv + attention + out-proj.

Sharding (hybrid, per the tensor-parallel hint): core c handles batch b = c//2
and head-group g = c%2 (8 of the 16 heads). Each core computes a partial
out-projection over its 8 heads; the host sums the two group partials per
batch and adds b_out.

Per-core kernel (mixed bf16/fp8 AV + 3-slot scores PSUM ring; 338us HW,
rel err 0.0129 vs the 2e-2 gate):

The steady state is EXP-ENGINE-bound, not PE-bound: the two scores matmuls
of a head pair run CONCURRENTLY on disjoint PE row halves (row-tiled
64-partition stationaries, ~3ns stagger, measured), so the PE only needs
~10us/unit while ACT+DVE carry ~16 chunk-exps (~24us combined). Every
choice below balances ACT vs DVE vs PE under that reality:
  - k-chunks are split 8/8 between the exp engines in ADJACENT PAIRS:
    ACT chunks do true exp -> bf16 E (no fp8-write penalty) and their AV
    runs as bf16 single-chunk matmuls on the under-used PE; DVE chunks do
    an int8 Schraudolph (i8 = trunc(psc + 56.03) bitcast e4m3 ~= exp(s/8);
    the 1.4427 scale is folded into the host-side q weights so it stays
    one tensor_scalar op) -> fp8e4 E, and their AV runs as fp8 DoubleRow
    pairs (one 512-col instruction contracts TWO 128-token k-chunks).
    Only half the E/V path is fp8, halving the quantization error.
  - V is split to match: bf16 V16 for ACT chunks, fp8 V8 (row stride 66 so
    every dual-fp8 ldweights slice lands on an even byte address - ISA
    restriction s3_lw_dual_fp8_restrictions) for DVE chunks. E tiles pack
    both dtypes in one [128,12,1024] bf16 ring slot (rows 8-11 hold the
    eight fp8 chunks two-per-row, accessed via bitcast views).
  - scores PSUM is a 3-deep ring ([128,1024] fp32 x 3 = 6 banks): with
    only 2 slots the psc recycle chains scores(c+2) <- exp(c) into two
    serial rails whose ~1.9us step time starves the PE (measured 455us);
    3 slots give three interleaved rails (~0.65us/chunk).
  - the ring is shared (same tag) with the prologue qk/v projections and
    the out-projection, which accumulate PAIRED 1024-col chains per psum
    tile and retire with ONE 1024-wide ACT/DVE copy (half the per-tile
    overhead, and no separate 2-bank ps_m pool). Prologue ordering: ALL
    xT readers (q-proj, v-proj) are emitted before unit-1 scores - E1
    recycles xT's big-pool slot, and a later xT reader would deadlock the
    shared-ring scheduler.
  - softmax normalize: ACT retires pw PSUM -> SBUF and computes the
    denominator reciprocal as ln + exp(-x) (the pinned act table has both;
    DVE's true reciprocal is an iterative 3.3us op and GPSIMD mixing
    tensor ops with partition_broadcast thrashes its Q7 ext-isa library,
    ~6us per MODIFY_POOL_CONFIG reload chain - both measured dead ends).
    GPSIMD only broadcasts the reciprocal; the waT mul stays on DVE.
    The pw-retire ACT copies are emitted right after the unit's last AV
    matmul (ahead of the late ACT exps in queue order) so the next unit's
    AV stream isn't gated ~2us late, and both heads' ln/exp recips run as
    single 1024-wide ops — the two changes COMPOSE (each alone measured
    neutral; together 344 -> 338us).
  - scores computed transposed (S^T[k, q] = kT.T @ qT) per 128-row k-chunk,
    with the two heads of a pair row-packed on disjoint PE row groups.
  - softmax denominator comes free as an all-ones column appended to V in
    the AV matmul (row 64 of the 65-row stationary).
  - the first 6 k-proj half-chains run contraction-outer across the 3 ring
    tiles so the PE tracks the arriving x DMA instead of idling per chunk.
  - qk-proj bias is applied by the ACT engine (Identity+bias PSUM->SBUF
    move, one 1024-wide op per (t,p,tt-pair)); the v-bias is folded EXACTLY
    into the host-side output constant (wa @ (v + bv) = wa @ v + bv since
    attention weights sum to 1).
  - out-proj staged in bf16 (halves the output DMA), its 4 token-chunks
    split across two emission points so the burst doesn't displace the
    whole psc ring; host sums the two head-group partials in fp32. The
    final q-range's out-proj partially accumulates during the last AV
    chain to shorten the tail.
"""

import os
import sys
from contextlib import ExitStack

import numpy as np

for _p in ("/opt/trn_rl_repo",):
    if _p not in sys.path and os.path.isdir(_p):
        sys.path.insert(0, _p)

import ml_dtypes

import concourse.bass as bass
import concourse.tile as tile
from concourse import bacc, mybir
from concourse.bass_utils import run_bass_kernel_spmd

BF16 = ml_dtypes.bfloat16
F32 = np.float32

D = 1024
H = 16
HD = 64
B = 4
N = 2048
NCORES = 8
G = 2  # head groups (tensor-parallel axis)
LH = H // G  # local heads per core
DC = D // 128  # 8 contraction chunks
KC = N // 128  # 16 k-token chunks
QT = N // 512  # 4 q tiles
TOK = N // 128  # 16 token chunks

# k-chunks whose exp runs on DVE (Schraudolph bit-trick) instead of ACT.
# Adjacent pairs, 8/8 split: DVE chunks are fp8 (int8 Schraudolph -> fp8e4
# DoubleRow AV pairs); ACT chunks write bf16 (no fp8-output penalty on the
# ACT exp) and their AV runs as bf16 singles on the under-used PE. This
# halves the E/V fp8 quantization error AND trims both exp engines.
DVE_SET = frozenset((2, 3, 6, 7, 10, 11, 14, 15))
# interleaved AV emission: bf16 singles + fp8 DoubleRow pairs, in kc order
AV_SEQ = (0, 1, (2, 3), 4, 5, (6, 7), 8, 9, (10, 11), 12, 13, (14, 15))
QSCALE = float(0.125 * 8.0 / np.log(2.0))  # folded into host q weights
EXP_B = 56.03  # e4m3 exponent bias*8 + rms-optimal shift (trunc-calibrated)
ACT_SCALE = float(0.125 / QSCALE)  # undo the fold for the true-exp path
VST = HD + 2  # V row stride (66): even so dual-fp8 ldweights slices align

_CACHE = {}


def _pin_act_tables():
    """Make the act-table chooser resolve exp AND ln to the one set that
    holds both (natural_log_exp_and_others), instead of thrashing between
    exp_and_others and natural_log on every softmax/reciprocal boundary
    (~1.3us ACT stall per reload). Other sets keep their index/id; we only
    hide exp/ln from them so they are never chosen for those funcs.
    """
    if _CACHE.get("act_pinned"):
        return
    from concourse import bacc as _bacc
    from concourse import hw_specs as _hw

    orig = _hw.get_activation_tables

    def patched(arch):
        t = dict(orig(arch))
        keep = "natural_log_exp_and_others"
        if keep in t:
            pinned = t[keep]
            t = {n: (s if n == keep else (s - pinned)) for n, s in t.items()}
        return t

    _hw.get_activation_tables = patched
    _bacc.get_activation_tables = patched
    _CACHE["act_pinned"] = True


def _build_nc():
    _pin_act_tables()
    nc = bacc.Bacc(None, target_bir_lowering=False)

    xT = nc.declare_dram_parameter("xT", [128, DC, N], mybir.dt.bfloat16, isOutput=False)
    # wqk[:, kc, 0, :] = k-features (4 pairs x 128), [:, kc, 1, :] = q-features
    wqk = nc.declare_dram_parameter("wqk", [128, DC, 2, 512], mybir.dt.bfloat16, isOutput=False)
    bqk = nc.declare_dram_parameter("bqk", [128, 8], mybir.dt.float32, isOutput=False)
    wv = nc.declare_dram_parameter("wv", [128, DC, LH * HD], mybir.dt.bfloat16, isOutput=False)
    wout = nc.declare_dram_parameter("wout", [128, LH * HD // 128, D], mybir.dt.bfloat16, isOutput=False)
    out = nc.declare_dram_parameter("out", [N, D], mybir.dt.bfloat16, isOutput=True)

    with tile.TileContext(nc) as tc, ExitStack() as ctx:
        const = ctx.enter_context(tc.tile_pool(name="const", bufs=1))
        big = ctx.enter_context(tc.tile_pool(name="big", bufs=4))
        work = ctx.enter_context(tc.tile_pool(name="work", bufs=1))
        outp = ctx.enter_context(tc.tile_pool(name="outp", bufs=2))
        small = ctx.enter_context(tc.tile_pool(name="small", bufs=2))
        # one shared PSUM ring: tag "sc" = 3 x [128,1024] fp32 (6 banks) for
        # scores + prologue projections + out-proj; tag "wa" on its own
        # pool = 2 x [65,512] (2 banks) for the AV accumulators.
        ps = ctx.enter_context(tc.tile_pool(name="ps", bufs=3, space="PSUM"))
        ps_wa = ctx.enter_context(tc.tile_pool(name="ps_wa", bufs=2, space="PSUM"))

        bqk_sb = const.tile([128, 8], mybir.dt.float32)
        # xT, wqk and wv share the 32KB big-pool ring with the E tiles:
        # they occupy slots during the projection prologue and are recycled
        # by E(1), E(2) and E(3) once their last readers retire.
        xT_sb = big.tile([128, DC, N], mybir.dt.bfloat16, tag="big", name="xT")
        wqk_sb = big.tile([128, DC, 2, 512], mybir.dt.bfloat16, tag="big", name="wqk")
        wv_sb = big.tile([128, DC, LH * HD], mybir.dt.bfloat16, tag="big", name="wv")
        nc.sync.dma_start(out=bqk_sb[:], in_=bqk[:])
        wout_sb = const.tile([128, LH * HD // 128, D], mybir.dt.bfloat16)
        # qkT[:, 0, p, :] = k-features of pair p; [:, 1, p, :] = q-features
        qkT_sb = work.tile([128, 2, 4, N], mybir.dt.bfloat16, tag="qkT")
        # V is split by exp engine: ACT chunks keep bf16 V (AV runs as bf16
        # singles), DVE chunks use fp8 V for the DoubleRow AV pairs. The fp8
        # row stride VST=66 keeps every dual-fp8 ldweights slice at an even
        # byte offset. Col HD holds the all-ones denominator column.
        V16 = work.tile([128, KC // 2, LH, HD + 1], mybir.dt.bfloat16, tag="V16")
        V8 = work.tile([128, KC // 4, 2, LH, VST], mybir.dt.float8e4, tag="V8")
        wa_pool = ctx.enter_context(tc.tile_pool(name="wa_pool", bufs=2))
        waT_ring = {}

        # ones column (index HD) for the free softmax denominator; the v-proj
        # copies below only fill [0:HD] so the column survives.
        nc.vector.memset(V16[:, :, :, HD : HD + 1], 1.0)
        nc.vector.memset(V8[:, :, :, :, HD : HD + 1], 1.0)

        def cidx(kc):
            """chunk -> index within its engine-set tiles (both sets take
            two chunks from every block of four)."""
            return (kc // 4) * 2 + (kc % 2)

        def E8pair(E, kc):
            """[128, 2, 1024] fp8 view of DVE chunks (kc, kc+1), kc%4==2."""
            return (
                E[:, 8 + cidx(kc) // 2, :]
                .bitcast(mybir.dt.float8e4)
                .rearrange("p (two k) -> p two k", two=2)
            )

        def emit_proj_pair(t, p, tt2):
            """Two [128 feats, 512 toks] tiles of the q/k projection sharing
            one [128,1024] psum tile and ONE 1024-wide ACT bias-copy.
            t=0 -> k-features, t=1 -> q-features of pair p, tok tiles tt2,tt2+1."""
            pq = ps.tile([128, 1024], mybir.dt.float32, tag="sc", name=f"pq_{t}_{p}_{tt2}")
            for half in range(2):
                tt = tt2 + half
                for kc in range(DC):
                    nc.tensor.matmul(
                        pq[:, half * 512 : (half + 1) * 512],
                        lhsT=wqk_sb[:, kc, t, p * 128 : (p + 1) * 128],
                        rhs=xT_sb[:, kc, tt * 512 : (tt + 1) * 512],
                        start=(kc == 0),
                        stop=(kc == DC - 1),
                    )
            nc.scalar.activation(
                out=qkT_sb[:, t, p, tt2 * 512 : (tt2 + 2) * 512],
                in_=pq[:],
                func=mybir.ActivationFunctionType.Identity,
                bias=bqk_sb[:, t * 4 + p : t * 4 + p + 1],
            )

        def emit_vproj():
            # v projection: V[tok, feat] = x @ w_v, two token-chunks per psum
            # tile, ONE 1024-wide DVE fp8 copy into the strided V layout.
            # The v-bias is EXACT as a host-side output constant (attention
            # weights sum to 1): folded into b_out on the host.
            for c2 in range(0, TOK, 2):
                pv = ps.tile([128, 1024], mybir.dt.float32, tag="sc", name=f"pv_{c2}")
                for half in range(2):
                    c = c2 + half
                    for kc in range(DC):
                        nc.tensor.matmul(
                            pv[:, half * 512 : (half + 1) * 512],
                            lhsT=xT_sb[:, kc, c * 128 : (c + 1) * 128],
                            rhs=wv_sb[:, kc, :],
                            start=(kc == 0),
                            stop=(kc == DC - 1),
                        )
                src = pv[:].rearrange("p (two l d) -> p two l d", two=2, l=LH)
                if c2 % 4 == 0:  # ACT-set chunks -> bf16 V
                    nc.vector.tensor_copy(
                        out=V16[:, cidx(c2) : cidx(c2) + 2, :, 0:HD], in_=src
                    )
                else:  # DVE-set chunks -> fp8 V
                    nc.vector.tensor_copy(out=V8[:, c2 // 4, :, :, 0:HD], in_=src)

        def unit(n):
            return n // 4, n % 4  # (q4, pair)

        def emit_scores_chunk(n, kc, E):
            q4, pair = unit(n)
            psc = ps.tile([128, 1024], mybir.dt.float32, tag="sc", name=f"sc_{n}_{kc}")
            for h01 in range(2):
                row = 64 * h01
                nc.tensor.matmul(
                    psc[:, h01 * 512 : (h01 + 1) * 512],
                    lhsT=qkT_sb[row : row + 64, 0, pair, kc * 128 : (kc + 1) * 128],
                    rhs=qkT_sb[row : row + 64, 1, pair, q4 * 512 : (q4 + 1) * 512],
                    start=True,
                    stop=True,
                )
            if kc in DVE_SET:
                # q-weights carry the 1.4427 scale: one add + int8 convert
                # IS exp (Schraudolph), bitcast to e4m3 via the packed view.
                j = cidx(kc)
                nc.vector.tensor_scalar_add(
                    out=E[:, 8 + j // 2, (j % 2) * 512 : (j % 2) * 512 + 512].bitcast(
                        mybir.dt.int8
                    ),
                    in0=psc[:],
                    scalar1=EXP_B,
                )
            else:
                nc.scalar.activation(
                    out=E[:, cidx(kc), :],
                    in_=psc[:],
                    func=mybir.ActivationFunctionType.Exp,
                    scale=ACT_SCALE,
                )

        def emit_av_item(n, item, E, pw, hs=(0, 1)):
            """One AV step per head: a bf16 single chunk (ACT-set) or an
            fp8 DoubleRow pair contracting chunks (kc, kc+1) in one 512-col
            stream (DVE-set). All steps accumulate one [65,512] chain."""
            _, pair = unit(n)
            if isinstance(item, tuple):
                kc = item[0]
                e8 = E8pair(E, kc)
                for h01 in hs:
                    nc.tensor.matmul(
                        pw[h01][:],
                        lhsT=V8[:, kc // 4, :, 2 * pair + h01, 0 : HD + 1],
                        rhs=e8[:, :, h01 * 512 : (h01 + 1) * 512],
                        start=False,
                        stop=(kc == KC - 2),
                        perf_mode=mybir.MatmulPerfMode.DoubleRow,
                    )
            else:
                kc = item
                for h01 in hs:
                    nc.tensor.matmul(
                        pw[h01][:],
                        lhsT=V16[:, cidx(kc), 2 * pair + h01, :],
                        rhs=E[:, cidx(kc), h01 * 512 : (h01 + 1) * 512],
                        start=(kc == 0),
                        stop=False,
                    )

        def emit_norm_copy(n, h01, pw):
            """Phase A1: the ACT copy that retires the pw PSUM slot into
            SBUF. Emitted RIGHT AFTER the unit's last AV matmul — ahead of
            the unit's last ACT exps in the ACT queue — so the next unit's
            AV stream isn't gated ~2us later than necessary."""
            pwS = small.tile([65, 512], mybir.dt.bfloat16, tag="pwS", name=f"pwS_{n}_{h01}")
            nc.scalar.activation(
                out=pwS[:], in_=pw[:], func=mybir.ActivationFunctionType.Copy
            )
            return pwS

        def emit_norm_recip(n, h01, pwS):
            """Phase A2 (late in the iteration, its consumer — the gpsimd
            broadcast — only runs next iteration): ln + exp(-x) reciprocal
            of the denominator row on the pinned act table."""
            lg = small.tile([1, 512], mybir.dt.bfloat16, tag="lg", bufs=1, name=f"lg_{n}_{h01}")
            nc.scalar.activation(
                out=lg[:], in_=pwS[64:65, :], func=mybir.ActivationFunctionType.Ln
            )
            recip = small.tile([1, 512], mybir.dt.bfloat16, tag="recip", name=f"r_{n}_{h01}")
            nc.scalar.activation(
                out=recip[:],
                in_=lg[:],
                func=mybir.ActivationFunctionType.Exp,
                scale=-1.0,
            )
            return recip

        def emit_norm_a(n, h01, pw):
            pwS = emit_norm_copy(n, h01, pw)
            return pwS, emit_norm_recip(n, h01, pwS)

        def emit_norm_copy2(n, pw):
            """Steady-state A1: both heads' pw retires into one [65,1024]
            staging (two ACT copies, still ahead of the late ACT exps)."""
            pwS = small.tile([65, 1024], mybir.dt.bfloat16, tag="pwS", name=f"pwS_{n}")
            for h01 in range(2):
                nc.scalar.activation(
                    out=pwS[:, h01 * 512 : (h01 + 1) * 512],
                    in_=pw[h01][:],
                    func=mybir.ActivationFunctionType.Copy,
                )
            return pwS

        def emit_norm_recip2(n, pwS):
            """Steady-state A2: ONE 1024-wide ln + ONE exp(-x) cover both
            heads' denominators (saves ~0.6us/unit of ACT small-op cost)."""
            lg = small.tile([1, 1024], mybir.dt.bfloat16, tag="lg", bufs=1, name=f"lg_{n}")
            nc.scalar.activation(
                out=lg[:], in_=pwS[64:65, :], func=mybir.ActivationFunctionType.Ln
            )
            recip = small.tile([1, 1024], mybir.dt.bfloat16, tag="recip", name=f"r_{n}")
            nc.scalar.activation(
                out=recip[:],
                in_=lg[:],
                func=mybir.ActivationFunctionType.Exp,
                scale=-1.0,
            )
            return recip

        def emit_norm_b_pb2(n, recip):
            rb = small.tile([64, 1024], mybir.dt.bfloat16, tag="rb", name=f"rb_{n}")
            nc.gpsimd.partition_broadcast(rb[:], recip[:])
            return rb

        def emit_norm_b_mul2(n, h01, pwS, rb):
            q4, pair = unit(n)
            row = 64 * h01
            nc.vector.tensor_mul(
                out=waT_ring[q4][row : row + 64, pair, :],
                in0=pwS[0:64, h01 * 512 : (h01 + 1) * 512],
                in1=rb[:, h01 * 512 : (h01 + 1) * 512],
            )

        def emit_norm_b_pb(n, h01, recip):
            """Phase B part 1 (next iteration): broadcast the reciprocal
            across 64 partitions on the otherwise-idle GPSIMD engine (frees
            the PE of 32 rank-1 matmuls and their stationary-swap drains)."""
            rb = small.tile([64, 512], mybir.dt.bfloat16, tag="rb", name=f"rb_{n}_{h01}")
            nc.gpsimd.partition_broadcast(rb[:], recip[:])
            return rb

        def emit_norm_b_mul(n, h01, pwS, rb):
            """Phase B part 2 (DVE): normalized waT = wa * (1/denom); both
            operands SBUF bf16 so the DVE runs in its 2x mode. NOT on
            GPSIMD: mixing tensor_tensor with partition_broadcast there
            makes every op swap the Q7 ext-isa library (~6us MODIFY_POOL_
            CONFIG reload chain, measured) which serializes the normalize
            path and stalls the whole pipeline."""
            q4, pair = unit(n)
            row = 64 * h01
            nc.vector.tensor_mul(
                out=waT_ring[q4][row : row + 64, pair, :],
                in0=pwS[0:64, :],
                in1=rb[:],
            )

        def emit_outproj(oq4, ccs=range(4)):
            # out projection for a finished q-range; overlaps the next
            # q-range's attention stream. Both 512-col halves accumulate in
            # one [128,1024] psum tile retired by ONE 1024-wide copy,
            # alternating ACT/DVE per token chunk; staged/stored in bf16.
            for cc in ccs:
                c = oq4 * 4 + cc
                o_sb = outp.tile([128, D], mybir.dt.bfloat16, tag="osb", name=f"o_{c}")
                po = ps.tile([128, 1024], mybir.dt.float32, tag="sc", name=f"po_{c}")
                for half in range(2):
                    for k4 in range(LH * HD // 128):
                        nc.tensor.matmul(
                            po[:, half * 512 : (half + 1) * 512],
                            lhsT=waT_ring[oq4][:, k4, cc * 128 : (cc + 1) * 128],
                            rhs=wout_sb[:, k4, half * 512 : (half + 1) * 512],
                            start=(k4 == 0),
                            stop=(k4 == LH * HD // 128 - 1),
                        )
                if cc % 2 == 0:
                    nc.scalar.activation(
                        out=o_sb[:], in_=po[:], func=mybir.ActivationFunctionType.Copy
                    )
                else:
                    nc.vector.tensor_copy(out=o_sb[:], in_=po[:])
                nc.sync.dma_start(out=out[c * 128 : (c + 1) * 128, :], in_=o_sb[:])

        # ---- prologue: all projections, then prime 3 units of scores ----
        # First 6 k-proj half-chains run contraction-OUTER across the 3 ring
        # tiles so each arriving xT chunk immediately feeds 6 matmuls and the
        # PE tracks the input DMA instead of idling ~1.4us per chunk.
        ko_t = [
            ps.tile([128, 1024], mybir.dt.float32, tag="sc", name=f"ko{i}")
            for i in range(3)
        ]
        ko_slots = [
            (ko_t[0][:, 0:512], 0, 0), (ko_t[0][:, 512:1024], 0, 1),
            (ko_t[1][:, 0:512], 0, 2), (ko_t[1][:, 512:1024], 0, 3),
            (ko_t[2][:, 0:512], 1, 0), (ko_t[2][:, 512:1024], 1, 1),
        ]
        # DMA issues interleaved with the matmuls that consume them, so each
        # chunk's matmuls gate on just that chunk's two transfers instead of
        # a coarse all-inputs semaphore threshold.
        for kc in range(DC):
            if kc == 0:
                # split the first chunk's transfers: the opening ko matmuls
                # need only the first half of xT chunk 0 + first cols of
                # wqk, so they start ~1.5us before the full chunk lands.
                nc.sync.dma_start(out=wqk_sb[:, 0, 0, 0:128], in_=wqk[:, 0, 0, 0:128])
                nc.sync.dma_start(out=xT_sb[:, 0, 0:1024], in_=xT[:, 0, 0:1024])
                nc.sync.dma_start(out=wqk_sb[:, 0, 0, 128:512], in_=wqk[:, 0, 0, 128:512])
                nc.sync.dma_start(out=xT_sb[:, 0, 1024:2048], in_=xT[:, 0, 1024:2048])
            else:
                nc.sync.dma_start(out=xT_sb[:, kc, :], in_=xT[:, kc, :])
                nc.sync.dma_start(out=wqk_sb[:, kc, 0, :], in_=wqk[:, kc, 0, :])
            for dst, p, tt in ko_slots:
                nc.tensor.matmul(
                    dst,
                    lhsT=wqk_sb[:, kc, 0, p * 128 : (p + 1) * 128],
                    rhs=xT_sb[:, kc, tt * 512 : (tt + 1) * 512],
                    start=(kc == 0),
                    stop=(kc == DC - 1),
                )
        for kc in range(DC):
            nc.sync.dma_start(out=wqk_sb[:, kc, 1, :], in_=wqk[:, kc, 1, :])
        nc.sync.dma_start(out=wv_sb[:], in_=wv[:])
        nc.sync.dma_start(out=wout_sb[:], in_=wout[:])
        # retire the ko tiles: each holds two token-tiles of one (t=0, p),
        # so one 1024-wide bias copy each.
        nc.scalar.activation(
            out=qkT_sb[:, 0, 0, 0:1024], in_=ko_t[0][:],
            func=mybir.ActivationFunctionType.Identity, bias=bqk_sb[:, 0:1],
        )
        nc.scalar.activation(
            out=qkT_sb[:, 0, 0, 1024:2048], in_=ko_t[1][:],
            func=mybir.ActivationFunctionType.Identity, bias=bqk_sb[:, 0:1],
        )
        nc.scalar.activation(
            out=qkT_sb[:, 0, 1, 0:1024], in_=ko_t[2][:],
            func=mybir.ActivationFunctionType.Identity, bias=bqk_sb[:, 1:2],
        )
        # remaining k-feature pairs, then the q-features units 0-7 need
        emit_proj_pair(0, 1, 2)
        emit_proj_pair(0, 2, 0)
        emit_proj_pair(0, 2, 2)
        emit_proj_pair(0, 3, 0)
        emit_proj_pair(0, 3, 2)

        E_tiles = {}

        def alloc_E(n):
            # rows 0-7: bf16 chunks (ACT set, by cidx); rows 8-11: the 8 fp8
            # chunks of the DVE set, packed two per bf16 row (see E8pair).
            E_tiles[n] = big.tile(
                [128, 12, 1024], mybir.dt.bfloat16, tag="big", name=f"E_{n}"
            )

        # Unit-0 scores (E0 takes a fresh ring slot) INTERLEAVED among the
        # q-proj pair chains: 16 consecutive priming chunks would stall the
        # PE ~0.4us per psc-ring recycle (exp latency with no filler,
        # measured ~6us cluster); each 16-matmul proj chain hides it. Unit 0
        # only needs q-features of pair 0 tile tt0 (the first chain below).
        # Units 1-2 still come after ALL remaining xT readers (q-proj +
        # vproj): E1 recycles xT's big-pool slot, so unit-1's exps gate on
        # xT's last readers — those must already be emitted or the shared
        # "sc" psum ring (proj tiles <- exp slot recycle) deadlocks the
        # scheduler.
        alloc_E(0)
        emit_proj_pair(1, 0, 0)
        sc0 = iter(range(KC))
        for p, tt2, nsc in (
            (1, 0, 2), (2, 0, 3), (3, 0, 3),
            (0, 2, 3), (1, 2, 3), (2, 2, 2), (3, 2, 0),
        ):
            for _ in range(nsc):
                emit_scores_chunk(0, next(sc0), E_tiles[0])
            emit_proj_pair(1, p, tt2)
        emit_vproj()  # last reader of xT; E(1) recycles its big-pool slot
        for n in (1, 2):
            alloc_E(n)
            for kc in range(KC):
                emit_scores_chunk(n, kc, E_tiles[n])

        # ---- steady state: AV(n) interleaved with scores/exp(n+3) ----
        # Emission order per iteration is tuned so every cross-engine gate
        # (pw-slot recycle, psc-slot recycle, E-slot recycle) is satisfied
        # ~an iteration before the PE reaches the dependent instruction.
        LEAD = 3
        norm_prev = None  # (pwS, recip) pairs of unit n-1
        for n in range(15):
            m = n + LEAD
            if m <= 15:
                alloc_E(m)
            pw = [
                ps_wa.tile([65, 512], mybir.dt.float32, tag="wa", name=f"wa_{n}_{h}")
                for h in range(2)
            ]
            # scores chunk 0 first: the iteration-opening AV gates on the
            # pw-slot handoff (ACT pwS copy at the end of iteration n-1),
            # so give ACT one matmul-pair of headroom.
            if m <= 15:
                emit_scores_chunk(m, 0, E_tiles[m])
            emit_av_item(n, AV_SEQ[0], E_tiles[n], pw)
            emit_av_item(n, AV_SEQ[1], E_tiles[n], pw)
            if m <= 15:
                for kc in (1, 2):
                    emit_scores_chunk(m, kc, E_tiles[m])
            emit_av_item(n, AV_SEQ[2], E_tiles[n], pw)
            pbs = None
            if n >= 1:
                if (n - 1) % 4 == 0:
                    waT_ring[(n - 1) // 4] = wa_pool.tile(
                        [128, LH * HD // 128, 512], mybir.dt.bfloat16,
                        tag="waT", name=f"waT_{(n - 1) // 4}",
                    )
                pbs = emit_norm_b_pb2(n - 1, norm_prev[1])
            for idx in range(9):  # AV items 3..11 with scores chunks 3..11
                if m <= 15:
                    emit_scores_chunk(m, 3 + idx, E_tiles[m])
                emit_av_item(n, AV_SEQ[3 + idx], E_tiles[n], pw)
                if idx == 1:
                    # muls into the DVE queue after the exp chunks that gate
                    # the psc recycle; outproj right after the muls it
                    # depends on, split across two loop positions so the
                    # 32-matmul burst doesn't displace the whole psc ring.
                    if pbs is not None:
                        for h01 in range(2):
                            emit_norm_b_mul2(n - 1, h01, norm_prev[0], pbs)
                    if n % 4 == 0 and n > 0:
                        emit_outproj(n // 4 - 1, ccs=(0, 1))
                if idx == 4 and n % 4 == 0 and n > 0:
                    emit_outproj(n // 4 - 1, ccs=(2, 3))
            if m <= 15:
                pass
            # pw retire copies BEFORE the remaining scores chunks: their
            # ACT exps (chunks 12, 13) would otherwise queue ahead of the
            # pwS copies and delay the next unit's whole AV stream by ~2us.
            pwSs = emit_norm_copy2(n, pw)
            if m <= 15:
                emit_scores_chunk(m, 12, E_tiles[m])
                emit_scores_chunk(m, 13, E_tiles[m])
                emit_scores_chunk(m, 14, E_tiles[m])
            norm_prev = (pwSs, emit_norm_recip2(n, pwSs))
            if m <= 15:
                emit_scores_chunk(m, KC - 1, E_tiles[m])
            del E_tiles[n]
        # ---- unit 15: h0 chain -> its normalize overlaps the h1 chain, so
        # outproj(3) only waits on the short h1 normalize tail ----
        bcs14 = emit_norm_b_pb2(14, norm_prev[1])
        pw = [
            ps_wa.tile([65, 512], mybir.dt.float32, tag="wa", name=f"wa_15_{h}")
            for h in range(2)
        ]
        waT_ring[3] = waT_ring.get(3) or wa_pool.tile(
            [128, LH * HD // 128, 512], mybir.dt.bfloat16, tag="waT", name="waT_3"
        )
        emit_av_item(15, AV_SEQ[0], E_tiles[15], pw, hs=(0,))
        emit_av_item(15, AV_SEQ[1], E_tiles[15], pw, hs=(0,))
        for h01 in range(2):
            emit_norm_b_mul2(14, h01, norm_prev[0], bcs14)
        for item in AV_SEQ[2:]:
            emit_av_item(15, item, E_tiles[15], pw, hs=(0,))
        n15_h0 = emit_norm_a(15, 0, pw[0])
        bc15_0 = emit_norm_b_pb(15, 0, n15_h0[1])
        emit_norm_b_mul(15, 0, n15_h0[0], bc15_0)
        for item in AV_SEQ:
            emit_av_item(15, item, E_tiles[15], pw, hs=(1,))
        # last normalize skips the pwS staging (nothing recycles pw after
        # this) so outproj's final matmuls wait ~1us less.
        lg15 = small.tile([1, 512], mybir.dt.bfloat16, tag="lg", bufs=1, name="lg_15_1")
        nc.scalar.activation(
            out=lg15[:], in_=pw[1][64:65, :], func=mybir.ActivationFunctionType.Ln
        )
        r15 = small.tile([1, 512], mybir.dt.bfloat16, tag="recip", name="r_15_1")
        nc.scalar.activation(
            out=r15[:], in_=lg15[:], func=mybir.ActivationFunctionType.Exp, scale=-1.0
        )
        rb15 = small.tile([64, 512], mybir.dt.bfloat16, tag="rb", name="rb_15_1")
        nc.gpsimd.partition_broadcast(rb15[:], r15[:])
        # partial outproj chains for the first two token chunks of q4=3:
        # pairs 0-2 are normalized, so k4 0..2 accumulate during the tail
        pos_part = {}
        for cc in (0, 1, 2):
            po = ps.tile([128, 1024], mybir.dt.float32, tag="sc", name=f"po_p_{cc}")
            for half in range(2):
                for k4 in range(3):
                    nc.tensor.matmul(
                        po[:, half * 512 : (half + 1) * 512],
                        lhsT=waT_ring[3][:, k4, cc * 128 : (cc + 1) * 128],
                        rhs=wout_sb[:, k4, half * 512 : (half + 1) * 512],
                        start=(k4 == 0),
                        stop=False,
                    )
            pos_part[cc] = po
        # cc3's half-0 partial goes in a freed "wa" PSUM bank (pw h0's slot
        # retired by norm_a(15,0); the 3 "sc" ring slots hold cc0-2): three
        # more k4=0-2 matmuls overlap the tail normalize chain.
        po3a = ps_wa.tile([128, 512], mybir.dt.float32, tag="wa", name="po3a")
        for k4 in range(3):
            nc.tensor.matmul(
                po3a[:],
                lhsT=waT_ring[3][:, k4, 3 * 128 : 4 * 128],
                rhs=wout_sb[:, k4, 0:512],
                start=(k4 == 0),
                stop=False,
            )
        nc.vector.tensor_mul(
            out=waT_ring[3][64:128, 3, :], in0=pw[1][0:64, :], in1=rb15[:]
        )
        del E_tiles[15]
        # finish the partial chains (k4=3 needs unit 15's normalize), then
        # the remaining chunks
        for cc in (0, 1, 2):
            c = 12 + cc
            o_sb = outp.tile([128, D], mybir.dt.bfloat16, tag="osb", name=f"o_{c}")
            po = pos_part[cc]
            for half in range(2):
                nc.tensor.matmul(
                    po[:, half * 512 : (half + 1) * 512],
                    lhsT=waT_ring[3][:, 3, cc * 128 : (cc + 1) * 128],
                    rhs=wout_sb[:, 3, half * 512 : (half + 1) * 512],
                    start=False,
                    stop=True,
                )
            if cc % 2 == 0:
                nc.scalar.activation(
                    out=o_sb[:], in_=po[:], func=mybir.ActivationFunctionType.Copy
                )
            else:
                nc.vector.tensor_copy(out=o_sb[:], in_=po[:])
            nc.sync.dma_start(out=out[c * 128 : (c + 1) * 128, :], in_=o_sb[:])
        # cc3: finish half-0's partial (1 matmul), run half-1's full chain,
        # then retire the halves on ACT and DVE CONCURRENTLY.
        o_sb15 = outp.tile([128, D], mybir.dt.bfloat16, tag="osb", name="o_15")
        nc.tensor.matmul(
            po3a[:],
            lhsT=waT_ring[3][:, 3, 3 * 128 : 4 * 128],
            rhs=wout_sb[:, 3, 0:512],
            start=False,
            stop=True,
        )
        po3b = ps.tile([128, 1024], mybir.dt.float32, tag="sc", name="po3b")
        for k4 in range(4):
            nc.tensor.matmul(
                po3b[:, 0:512],
                lhsT=waT_ring[3][:, k4, 3 * 128 : 4 * 128],
                rhs=wout_sb[:, k4, 512:1024],
                start=(k4 == 0),
                stop=(k4 == 3),
            )
        nc.scalar.activation(
            out=o_sb15[:, 0:512], in_=po3a[:], func=mybir.ActivationFunctionType.Copy
        )
        nc.vector.tensor_copy(out=o_sb15[:, 512:1024], in_=po3b[:, 0:512])
        nc.sync.dma_start(out=out[15 * 128 : 16 * 128, :], in_=o_sb15[:])

    nc.compile()
    return nc


def _prep_in_maps(x, w_qkv, b_qkv, w_out):
    """Host-side shard + relayout. Core c -> (batch c//2, head-group c%2)."""
    wq = w_qkv[:, :D].reshape(D, H, HD)
    wk = w_qkv[:, D : 2 * D].reshape(D, H, HD)
    wv_ = w_qkv[:, 2 * D :].reshape(D, H, HD)
    bq = b_qkv[:D].reshape(H, HD)
    bk = b_qkv[D : 2 * D].reshape(H, HD)
    bv = b_qkv[2 * D :].reshape(H, HD)
    wo = w_out.reshape(H, HD, D)

    per_group = {}
    for g in range(G):
        h0 = g * LH
        # feature order: block t=0 = k feats, t=1 = q feats (scaled by
        # QSCALE so the DVE bit-trick exp needs no multiply); within a
        # block, pair p occupies cols p*128..(p+1)*128 (first head at 0-63).
        Wqk = np.empty((D, 2, 4, 128), F32)
        Bqk = np.empty((2, 4, 128), F32)
        for p in range(LH // 2):
            ha, hb = h0 + 2 * p, h0 + 2 * p + 1
            Wqk[:, 0, p, 0:64] = wk[:, ha]
            Wqk[:, 0, p, 64:128] = wk[:, hb]
            Wqk[:, 1, p, 0:64] = wq[:, ha] * QSCALE
            Wqk[:, 1, p, 64:128] = wq[:, hb] * QSCALE
            Bqk[0, p, 0:64] = bk[ha]
            Bqk[0, p, 64:128] = bk[hb]
            Bqk[1, p, 0:64] = bq[ha] * QSCALE
            Bqk[1, p, 64:128] = bq[hb] * QSCALE
        wqk_arr = np.ascontiguousarray(
            Wqk.reshape(DC, 128, 2, 512).transpose(1, 0, 2, 3)
        ).astype(BF16)
        bqk_arr = np.ascontiguousarray(Bqk.reshape(8, 128).T)

        Wv = wv_[:, h0 : h0 + LH, :].reshape(D, LH * HD)
        wv_arr = np.ascontiguousarray(
            Wv.reshape(DC, 128, LH * HD).transpose(1, 0, 2)
        ).astype(BF16)

        Wo = wo[h0 : h0 + LH].reshape(LH * HD, D)
        wout_arr = np.ascontiguousarray(
            Wo.reshape(LH * HD // 128, 128, D).transpose(1, 0, 2)
        ).astype(BF16)
        per_group[g] = (wqk_arr, bqk_arr, wv_arr, wout_arr)

    in_maps = []
    for c in range(NCORES):
        b, g = divmod(c, G)
        wqk_arr, bqk_arr, wv_arr, wout_arr = per_group[g]
        xT_arr = np.ascontiguousarray(
            x[b].T.reshape(DC, 128, N).transpose(1, 0, 2)
        ).astype(BF16)
        in_maps.append(
            {
                "xT": xT_arr,
                "wqk": wqk_arr,
                "bqk": bqk_arr,
                "wv": wv_arr,
                "wout": wout_arr,
            }
        )
    return in_maps


def _ensure_ntff_hook():
    """Register the axon NTFF profile hook if the image's antenv lacks it.

    Mirrors trn_agent_boot.trn_boot._ntff_profile_via_ctypes: drives NRT
    profiling through the injected libaxon_pjrt.so C ABI. Without this,
    run_bass_kernel_spmd(trace=True) raises ImportError under axon.
    """
    try:
        from antenv.axon_hooks import get_axon_ntff_profile_hook  # noqa: F401

        return
    except ImportError:
        pass

    import contextlib
    import ctypes
    import types

    so_path = "/opt/axon/libaxon_pjrt.so"
    lib = ctypes.CDLL(so_path)
    if not hasattr(lib, "axon_start_nrt_profile"):
        return
    lib.axon_start_nrt_profile.argtypes = [ctypes.POINTER(ctypes.c_int64), ctypes.c_size_t]
    lib.axon_start_nrt_profile.restype = ctypes.c_int64
    lib.axon_stop_nrt_profile.argtypes = [ctypes.c_char_p]
    lib.axon_stop_nrt_profile.restype = ctypes.c_int64

    @contextlib.contextmanager
    def _hook(output_dir, device_ids):
        import jax

        jax.devices()
        if device_ids:
            ids = (ctypes.c_int64 * len(device_ids))(*device_ids)
            rc = lib.axon_start_nrt_profile(ids, len(device_ids))
        else:
            rc = lib.axon_start_nrt_profile(None, 0)
        if rc != 0:
            raise RuntimeError(f"axon_start_nrt_profile rc={rc}")
        try:
            yield
        finally:
            n = lib.axon_stop_nrt_profile(str(output_dir).encode())
            print(f"ntff profile: {n} file(s) written to {output_dir}", file=sys.stderr)

    mod = types.ModuleType("antenv.axon_hooks")
    mod.get_axon_ntff_profile_hook = lambda: _hook
    sys.modules["antenv.axon_hooks"] = mod

    # No artifact bucket in this sandbox; keep the NEFF dir local.
    from concourse import bass_utils as _bu

    _bu.upload_artifacts = lambda tmpdir: tmpdir


def kernel(x, w_qkv, b_qkv, w_out, b_out):
    x = np.asarray(x, dtype=F32)
    w_qkv = np.asarray(w_qkv, dtype=F32)
    b_qkv = np.asarray(b_qkv, dtype=F32)
    w_out = np.asarray(w_out, dtype=F32)
    b_out = np.asarray(b_out, dtype=F32)

    if "nc" not in _CACHE:
        _CACHE["nc"] = _build_nc()
    nc = _CACHE["nc"]

    in_maps = _prep_in_maps(x, w_qkv, b_qkv, w_out)
    trace = bool(int(os.environ.get("BASSMHA_TRACE", "0")))
    kwargs = {}
    if trace:
        _ensure_ntff_hook()
        tdir = os.environ.get("BASSMHA_TRACE_DIR")
        if tdir:
            os.makedirs(tdir, exist_ok=True)
            kwargs["tmpdir"] = tdir
    res = run_bass_kernel_spmd(nc, in_maps, list(range(NCORES)), trace=trace, **kwargs)
    _CACHE["last_results"] = res

    # v-bias folds exactly into an output constant: wa @ (v + bv) =
    # wa @ v + bv because the attention weights sum to 1.
    const_add = b_out + b_qkv[2 * D :].astype(F32) @ w_out
    out = np.empty((B, N, D), F32)
    for b in range(B):
        out[b] = res.results[2 * b]["out"].astype(F32)
        out[b] += res.results[2 * b + 1]["out"].astype(F32)
        out[b] += const_add
    return out
